# revision 30
# baseline (speedup 1.0000x reference)
"""Trainium2 Bass kernel for windowed 3D attention (sparse_attention).

Per window (256 windows on a 16x16 grid): N=294 tokens, d=256, 8 heads x 32.
qkv = x @ w_qkv.T ; A = softmax(q k^T/sqrt(dh) + bias) ; out = (A v) @ w_out.T

Sharding: data-parallel over the grid; core s takes X-rows [2s, 2s+2) = 32 windows.

Device strategy v3 (engine-balanced, PE kept warm):
  - all matmul inputs bf16 (host-cast); 1/sqrt(dh) folded into Wq on host
  - Q^T/K^T d-major; V token-major
  - S^T[j,i] f32 PSUM in head-pair tiles [128,2,512] (one bank per head);
    ps_s bufs=2 pipelines S-mms against ScalarE's exp ACT (one per pair)
  - bias: all heads identity-seeded into PSUM (exact add pre-exp);
    the full-array seed matmuls double as HAM-activity fuel, keeping
    the PE clock at 2.4 GHz
  - rowsums: ones[jn,32] lhsT matmuls 4-way col-tiled -> 32-row broadcast
    per head for free; reciprocal_approx_fast on VectorE
  - O^T: V^T A^T col-tiled 4-way, f32 PSUM accumulated over j-chunks;
    normalized during PSUM->SBUF evac (VectorE tensor_tensor with rinv)
  - Y^T = w_out^T.T @ O_norm^T d-major -> DMA out [D, TOK]; host transposes
"""

import os
from contextlib import ExitStack

import numpy as np
import ml_dtypes

import concourse.bass as bass
import concourse.mybir as mybir
import concourse.tile as tile
from concourse import bacc
from concourse.bass_utils import run_bass_kernel_spmd
from concourse.masks import make_identity

F32 = mybir.dt.float32
BF16 = mybir.dt.bfloat16

L, W, D, H = 6, 7, 256, 8
DH = D // H                      # 32
N = L * W * W                    # 294
GX = GY = 16
NCORES = 8
XPC = GX // NCORES               # X-rows per core
NW = XPC * GY                    # 32 windows per core
TOK = NW * N                     # 9408 tokens per core
SCALE = DH ** -0.5
NS = 512 - N                     # 218: odd-head split point (bank boundary)

CH = [(0, 128), (128, 128), (256, 38)]    # j / i chunks

TRACE = False     # set by test.py for profiling runs
_CACHE = {}


def _body(ctx, tc, xT, wqkvT, woutT, biasT, expBT, y):
    nc = tc.nc

    const = ctx.enter_context(tc.tile_pool(name="const", bufs=1))
    xpool = ctx.enter_context(tc.tile_pool(name="xin", bufs=6))
    qkpool = ctx.enter_context(tc.tile_pool(name="qk", bufs=4))
    vpool = ctx.enter_context(tc.tile_pool(name="vtok", bufs=4))
    apool = ctx.enter_context(tc.tile_pool(name="at", bufs=30))
    ampool = ctx.enter_context(tc.tile_pool(name="am", bufs=24))
    rvpool = ctx.enter_context(tc.tile_pool(name="rinv", bufs=3))
    otpool = ctx.enter_context(tc.tile_pool(name="ot", bufs=3))
    ypool = ctx.enter_context(tc.tile_pool(name="yout", bufs=3))

    ps_s = ctx.enter_context(tc.tile_pool(name="ps_s", bufs=2, space="PSUM"))
    ps_rso = ctx.enter_context(tc.tile_pool(name="ps_rso", bufs=1, space="PSUM"))
    ps_m = ctx.enter_context(tc.tile_pool(name="ps_m", bufs=2, space="PSUM"))

    # ---- resident constants ----
    wqkv_s = const.tile([128, 2, 2 * D], BF16)     # Q^T,K^T weight cols (q pre-scaled)
    nc.sync.dma_start(out=wqkv_s, in_=wqkvT.rearrange("(c p) n -> p c n", c=2)[:, :, 0:2 * D])
    wv_s = const.tile([128, 2, D], BF16)
    nc.sync.dma_start(out=wv_s, in_=wqkvT.rearrange("(c p) n -> p c n", c=2)[:, :, 2 * D:3 * D])
    wout_s = const.tile([128, 2, D], BF16)
    nc.sync.dma_start(out=wout_s, in_=woutT.rearrange("(c p) n -> p c n", c=2))
    # raw bias^T for seeded heads 0-1: [j, 2*N]; exp(bias)^T for heads 2-7
    # big bias tables go via SWDGE so window 0's x-load isn't queued behind them
    bias_s = const.tile([128, 3, 2 * N], BF16)
    for jc, (j0, jn) in enumerate(CH):
        nc.gpsimd.dma_start(out=bias_s[:jn, jc, :], in_=biasT[j0:j0 + jn, :])
    expb_s = const.tile([128, 3, 6 * N], BF16)
    for jc, (j0, jn) in enumerate(CH):
        nc.gpsimd.dma_start(out=expb_s[:jn, jc, :], in_=expBT[j0:j0 + jn, :])
    ident_b = const.tile([128, 128], BF16)
    make_identity(nc, ident_b)
    ones_b = const.tile([128, 32], BF16)
    nc.vector.memset(ones_b, 1.0)
    # warm the exp spline table while const DMAs stream, so the first real
    # exp ACT doesn't stall ~2.7us on ACT_TABLE_LOAD
    wrm = const.tile([1, 2], F32)
    nc.vector.memset(wrm, 0.0)
    nc.scalar.activation(wrm[0:1, 0:1], wrm[0:1, 1:2],
                         mybir.ActivationFunctionType.Exp)

    def emit_qkv(w):
        """Create qk/vtok tiles for window w; return filler closures that emit
        the full-array matmuls (HAM activity fuel, interleaved with S quads)."""
        t0 = w * N
        xw = xpool.tile([128, 2, N], BF16, tag="xw", name=f"xw{w}")
        nc.sync.dma_start(out=xw, in_=xT.rearrange("(c p) t -> p c t", c=2)[:, :, t0:t0 + N])
        qk = qkpool.tile([128, 4, N], BF16, tag="qk", name=f"qk{w}")
        vtok = vpool.tile([128, 3, D], BF16, tag="vt", name=f"vt{w}")
        fills = []
        pq_box = {}
        for m in range(4):
            def f_qk_a(m=m, xw=xw, w=w):
                pq = ps_m.tile([128, 512], F32, tag="psmisc", name=f"pq{w}_{m}")
                pq_box[m] = pq
                nc.tensor.matmul(
                    pq[:, :N], wqkv_s[:, 0, m * 128:(m + 1) * 128], xw[:, 0, :],
                    start=True, stop=False)
            def f_qk_b(m=m, xw=xw, qk=qk):
                pq = pq_box.pop(m)
                nc.tensor.matmul(
                    pq[:, :N], wqkv_s[:, 1, m * 128:(m + 1) * 128], xw[:, 1, :],
                    start=False, stop=True)
                nc.vector.tensor_copy(qk[:, m, :], pq[:, :N])
            fills.append(f_qk_a)
            fills.append(f_qk_b)
        pv_box = {}
        def f_v_a(xw=xw, w=w):
            pv = ps_m.tile([128, 512], F32, tag="psmisc", name=f"pv{w}_01")
            pv_box[0] = pv
            for kc in range(2):
                nc.tensor.matmul(
                    pv[:, 0:D], xw[:, kc, 0:128], wv_s[:, kc, :],
                    start=(kc == 0), stop=False)
        def f_v_b(xw=xw, vtok=vtok):
            pv = pv_box.pop(0)
            for kc in range(2):
                nc.tensor.matmul(
                    pv[:, D:2 * D], xw[:, kc, 128:256], wv_s[:, kc, :],
                    start=False, stop=(kc == 1))
            nc.vector.tensor_copy(
                vtok[:, 0:2, :], pv[:, :].rearrange("p (a b) -> p a b", a=2))
        def f_v_c(xw=xw, vtok=vtok, w=w):
            pv = ps_m.tile([128, 512], F32, tag="psmisc", name=f"pv{w}_2")
            for kc in range(2):
                nc.tensor.matmul(
                    pv[:38, :D], xw[:, kc, 256:294], wv_s[:, kc, :],
                    start=(kc == 0), stop=(kc == 1))
            nc.vector.tensor_copy(vtok[:38, 2, :], pv[:38, :D])
        fills.extend([f_v_a, f_v_b, f_v_c])
        return qk, vtok, fills

    carry = []
    # window 0: emit its QK/V upfront
    qk, vtok, fills0 = emit_qkv(0)
    for f in fills0:
        f()

    for w in range(NW):
        t0 = w * N
        # prefetch + prepare next window's QK/V as interleave fillers
        if w + 1 < NW:
            qk_n, vt_n, fillers = emit_qkv(w + 1)
        else:
            qk_n, vt_n, fillers = None, None, []

        # ---- rowsum + O^T emitter (g0 interleaves into the S loop) ----
        ot = otpool.tile([128, 2, N], BF16, tag="ot", name=f"ot{w}")
        rinv = rvpool.tile([128, 2, N], F32, tag="rv", name=f"rv{w}")

        rso_box = {}

        def rso_chunk(g, jc, w=w, aslb=None, vtokb=None):
            asl, vtok = aslb, vtokb
            if jc == 0:
                rso_box[g] = ps_rso.tile([128, 2, 512], F32, tag="rso",
                                         name=f"rso{w}_{g}")
            prso = rso_box[g]
            j0, jn = CH[jc]
            for c in range(4):
                nc.tensor.matmul(
                    prso[32 * c:32 * c + 32, 0, :N],
                    ones_b[:jn, :], asl[jc][4 * g + c],
                    start=(jc == 0), stop=(jc == 2),
                    tile_position=(0, 32 * c), skip_group_check=True)
            for c in range(4):
                h = 4 * g + c
                nc.tensor.matmul(
                    prso[32 * c:32 * c + 32, 1, :N],
                    vtok[:jn, jc, 32 * h:32 * h + 32], asl[jc][h],
                    start=(jc == 0), stop=(jc == 2),
                    tile_position=(0, 32 * c), skip_group_check=True)

        def rso_fin(g, ot=ot, rinv=rinv):
            prso = rso_box.pop(g)
            nc.vector.reciprocal_approx_fast(rinv[:, g, :], prso[:, 0, :N])
            nc.vector.tensor_tensor(
                out=ot[:, g, :], in0=prso[:, 1, :N], in1=rinv[:, g, :],
                op=mybir.AluOpType.mult)

        def emit_rso(g, aslb=None, vtokb=None):
            for jc in range(3):
                rso_chunk(g, jc, aslb=aslb, vtokb=vtokb)
            rso_fin(g)

        # ---- S^T per (jc, head-pair); exp -> A^T bf16; bias mix ----
        # asl[jc][h]: AP of final A^T slice [jn, N] for head h
        asl = [[None] * 8 for _ in range(3)]
        blk = 0
        rs_emitted = False
        for phalf in ((0, 1), (2, 3)):
          for jc, (j0, jn) in enumerate(CH):
            for p in phalf:
                blk += 1
                if blk == 3 and carry:
                    fillers[4:4] = carry      # splice deferred rso/Y mid-queue
                    carry = []
                if fillers:
                    fillers.pop(0)()
                elif w == NW - 1 and blk in (9, 11):
                    # wind-down: start the last window's own rso-g1 early
                    rso_chunk(1, (blk - 9) // 2, aslb=asl, vtokb=vtok)
                if phalf == (2, 3) and not rs_emitted:
                    rs_emitted = True
                    emit_rso(0, aslb=asl, vtokb=vtok)
                he, ho = 2 * p, 2 * p + 1
                be, bo = 32 * (he % 4), 32 * (ho % 4)
                ps2 = ps_s.tile([128, 2, 512], F32, tag="s2", name=f"s2_{w}_{jc}_{p}")
                sfl = ps2.rearrange("p a b -> p (a b)")      # flat [128, 1024]
                # has_written semantics: start=True clears the WHOLE bank's
                # bits -> exactly one start=True per bank (its first write);
                # later writes use start=False (overwrite-where-clear).
                seeded = p < 1
                if seeded:
                    nc.tensor.matmul(
                        sfl[:jn, 0:N], ident_b[:jn, :jn],
                        bias_s[:jn, jc, he * N:he * N + N],
                        start=True, stop=False, skip_group_check=True)
                    nc.tensor.matmul(
                        sfl[:jn, 512:512 + N], ident_b[:jn, :jn],
                        bias_s[:jn, jc, ho * N:ho * N + N],
                        start=True, stop=False, skip_group_check=True)
                # S quads (even head: 1 mm; odd head: split at bank boundary)
                nc.tensor.matmul(
                    sfl[:jn, 0:N],
                    qk[be:be + 32, 2 + he // 4, j0:j0 + jn],
                    qk[be:be + 32, he // 4, :],
                    start=not seeded, stop=True, tile_position=(be, 0),
                    skip_group_check=True)
                nc.tensor.matmul(
                    sfl[:jn, 512:512 + N],
                    qk[bo:bo + 32, 2 + ho // 4, j0:j0 + jn],
                    qk[bo:bo + 32, ho // 4, :],
                    start=not seeded, stop=True, tile_position=(bo, 0),
                    skip_group_check=True)
                a_t = apool.tile([128, 2 * N], BF16, tag="at", name=f"at{w}_{jc}_{p}")
                nc.scalar.activation(
                    a_t[:jn, :].rearrange("p (a b) -> p a b", a=2), ps2[:jn, :, :N],
                    mybir.ActivationFunctionType.Exp)
                if seeded:
                    asl[jc][he] = a_t[:jn, 0:N]
                    asl[jc][ho] = a_t[:jn, N:2 * N]
                else:
                    # bias multiply: pairs 1-2 on VectorE, pair 3 on GpSimd
                    a_m = ampool.tile([128, 2 * N], BF16, tag="am", name=f"am{w}_{jc}_{p}")
                    if p < 3:
                        nc.vector.tensor_tensor(
                            out=a_m[:jn, :], in0=a_t[:jn, :],
                            in1=expb_s[:jn, jc, (he - 2) * N:(he - 2) * N + 2 * N],
                            op=mybir.AluOpType.mult)
                    else:
                        # split the last pair: head 6 on Vector, head 7 on GpSimd
                        nc.vector.tensor_tensor(
                            out=a_m[:jn, 0:N], in0=a_t[:jn, 0:N],
                            in1=expb_s[:jn, jc, (he - 2) * N:(he - 2) * N + N],
                            op=mybir.AluOpType.mult)
                        nc.gpsimd.tensor_tensor(
                            out=a_m[:jn, N:2 * N], in0=a_t[:jn, N:2 * N],
                            in1=expb_s[:jn, jc, (ho - 2) * N:(ho - 2) * N + N],
                            op=mybir.AluOpType.mult)
                    asl[jc][he] = a_m[:jn, 0:N]
                    asl[jc][ho] = a_m[:jn, N:2 * N]

        def emit_y(w=w, t0=t0, ot=ot):
            ysb = ypool.tile([128, 2, N], F32, tag="ysb", name=f"ysb{w}")
            pyt = ps_rso.tile([128, 2, 512], F32, tag="rso", name=f"py{w}")
            for m in range(2):
                for g in range(2):
                    nc.tensor.matmul(
                        pyt[:, m, :N], wout_s[:, g, m * 128:(m + 1) * 128], ot[:, g, :],
                        start=(g == 0), stop=(g == 1))
            nc.vector.tensor_copy(ysb[:, :, :], pyt[:, :, :N])
            nc.scalar.dma_start(
                out=y.rearrange("(c p) t -> p c t", c=2)[:, :, t0:t0 + N], in_=ysb)

        import functools
        if w == NW - 1:
            carry = [functools.partial(rso_chunk, 1, 2, aslb=asl, vtokb=vtok),
                     functools.partial(rso_fin, 1), emit_y]
        else:
            carry = [functools.partial(rso_chunk, 1, jc, aslb=asl, vtokb=vtok)
                     for jc in range(3)]
            carry += [functools.partial(rso_fin, 1), emit_y]
        for f in fillers:      # any leftover next-window fillers
            f()
        qk, vtok = qk_n, vt_n
    for f in carry:            # flush last window's deferred rso-g1 + Y
        f()


def _build():
    if "nc" in _CACHE:
        return _CACHE["nc"]
    nc = bacc.Bacc("TRN2", target_bir_lowering=False)
    xT = nc.dram_tensor("xT", [D, TOK], BF16, kind="ExternalInput").ap()
    wqkvT = nc.dram_tensor("wqkvT", [D, 3 * D], BF16, kind="ExternalInput").ap()
    woutT = nc.dram_tensor("woutT", [D, D], BF16, kind="ExternalInput").ap()
    biasT = nc.dram_tensor("biasT", [N, 2 * N], BF16, kind="ExternalInput").ap()
    expBT = nc.dram_tensor("expBT", [N, 6 * N], BF16, kind="ExternalInput").ap()
    y = nc.dram_tensor("y", [D, TOK], F32, kind="ExternalOutput").ap()
    with tile.TileContext(nc) as tc, ExitStack() as ctx:
        _body(ctx, tc, xT, wqkvT, woutT, biasT, expBT, y)
    nc.compile()
    _CACHE["nc"] = nc
    return nc


def kernel(x, w_qkv, w_out, bias_table, rel_idx):
    x = np.asarray(x, dtype=np.float32)
    w_qkv = np.asarray(w_qkv, dtype=np.float32)
    w_out = np.asarray(w_out, dtype=np.float32)
    bias_table = np.asarray(bias_table, dtype=np.float32)
    rel_idx = np.asarray(rel_idx)

    # host-side layout prep
    # x[0]: [l, X, Y, w1, w2, d] -> xT [d, (X Y l w1 w2)] bf16
    xt = np.ascontiguousarray(
        x[0].transpose(5, 1, 2, 0, 3, 4)).reshape(D, GX * GY * N).astype(ml_dtypes.bfloat16)
    wq = w_qkv.copy()
    wq[:D] *= SCALE                        # fold attention scale into Wq
    wqkvT = np.ascontiguousarray(wq.T).astype(ml_dtypes.bfloat16)
    woutT = np.ascontiguousarray(w_out.T).astype(ml_dtypes.bfloat16)
    bias = bias_table[rel_idx]             # [i, j, h]
    biasT_full = bias.transpose(1, 2, 0)   # [j, h, i]
    biasT = np.ascontiguousarray(biasT_full[:, 0:2, :]).reshape(N, 2 * N).astype(ml_dtypes.bfloat16)
    expBT = np.ascontiguousarray(np.exp(biasT_full[:, 2:8, :])).reshape(N, 6 * N).astype(ml_dtypes.bfloat16)

    nc = _build()
    in_maps = []
    for s in range(NCORES):
        xs = np.ascontiguousarray(xt[:, s * TOK:(s + 1) * TOK])
        in_maps.append({"xT": xs, "wqkvT": wqkvT, "woutT": woutT,
                        "biasT": biasT, "expBT": expBT})

    res = run_bass_kernel_spmd(nc, in_maps, core_ids=list(range(NCORES)), trace=TRACE)
    _CACHE["res"] = res
    if TRACE and res.exec_time_ns is not None:
        print(f"HW exec time: {res.exec_time_ns} ns")
        _CACHE["exec_time_ns"] = res.exec_time_ns

    # gather: per-core y [D, 9408] d-major -> [1, l, X, Y, w1, w2, d]
    out = np.empty((1, L, GX, GY, W, W, D), dtype=np.float32)
    for s in range(NCORES):
        yc = res.results[s]["y"].reshape(D, XPC, GY, L, W, W)
        out[0, :, s * XPC:(s + 1) * XPC] = yc.transpose(3, 1, 2, 4, 5, 0)
    return out
